# revision 15
# baseline (speedup 1.0000x reference)
"""Trainium2 Bass kernel for a sparse (sliding-window) attention layer.

Reference computation (B=2, S=2048, D=2048, H=16 heads, window=256, fp32):
    qp = q @ Wq + bq ; kp = k @ Wk + bk ; vp = v @ Wv + bv
    per-head scores with mask (0 <= q_idx - k_idx <= 256), softmax, ctx
    out = merge_heads(ctx) @ Wo + bo
    returns (out, kp, vp)

Sharding: 8 cores = 2 (batch) x 4 (head groups of 4 heads / 512 dims).
Each core computes its batch's projections for its 512 output dims,
windowed attention for its 4 heads, and a partial out-projection (rows
of Wo owned by its heads).  Host sums the 4 bf16 partial outputs per
batch and concatenates kp/vp slices.

Attention is computed in transposed form: S^T[k, q] = kpT^T @ qpT per
key block (one N=384 matmul covering the 3 query blocks in this key
block's window), exp on the scalar engine, triangular masking with
gpsimd affine_select (zero-fill), and PV as ctx[q, dh] with lhsT =
exp^T blocks; a ones-column appended to v yields the softmax row sums
in the same matmul.  This removes all probability transposes of the
previous design; only one [128,128] ctx transpose per query block
remains (to feed the out-projection).

Matmul operands are bf16 (fp32 PSUM accumulation); outputs are written
bf16 and widened on host (tolerance 2e-2 >> bf16 rounding).
"""

import os

import numpy as np

B = 2
S = 2048
D = 2048
GD = 512          # dims per core (4 heads x 128)
NH = 4            # heads per core
P = 128
WIN = 256         # sliding window
NDB = D // P      # 16 contraction blocks
SC = 512          # seq chunk for projections
NSC = S // SC     # 4
NSB = S // P      # 16 seq blocks
NRING = 8         # exp ring slots per head
SCALE = 1.0 / np.sqrt(P)

_CACHE = {}
LAST_RESULTS = None

# attention blocks that become computable after q-projection chunk c:
# scores(kb) needs qpT blocks kb..kb+2 ; PV(qb) needs exp(kb=qb) => qb+2
_KB_BATCH = [range(0, 2), range(2, 6), range(6, 10), range(10, 16)]
_QB_BATCH = [range(0, 2), range(2, 6), range(6, 10), range(10, 16)]


def _build_nc():
    import sys
    sys.path.insert(0, "/opt/trn_rl_repo")
    import concourse.bass as bass  # noqa: F401
    import concourse.tile as tile
    from concourse import mybir, bacc
    from concourse.masks import make_identity
    from contextlib import ExitStack

    F32 = mybir.dt.float32
    CDT = mybir.dt.bfloat16
    AluOp = mybir.AluOpType
    ActFn = mybir.ActivationFunctionType

    nc = bacc.Bacc("TRN2", target_bir_lowering=False, debug=False, num_devices=8)

    # host-packed inputs: x*_c[(c p), (do s)] = x^T[do*128+p, c*512+s]
    xq_c = nc.dram_tensor("xq_c", [NSC * P, NDB * SC], CDT, kind="ExternalInput")
    xk_c = nc.dram_tensor("xk_c", [NSC * P, NDB * SC], CDT, kind="ExternalInput")
    xv_c = nc.dram_tensor("xv_c", [NSC * P, NDB * SC], CDT, kind="ExternalInput")
    # w*_c[p, (do n)] = W[do*128+p, g*512+n]
    wq_c = nc.dram_tensor("wq_c", [P, NDB * GD], CDT, kind="ExternalInput")
    wk_c = nc.dram_tensor("wk_c", [P, NDB * GD], CDT, kind="ExternalInput")
    wv_c = nc.dram_tensor("wv_c", [P, NDB * GD], CDT, kind="ExternalInput")
    # wo_c[p, (h n)] = Wo[g*512 + h*128 + p, n]
    wo_c = nc.dram_tensor("wo_c", [P, NH * D], CDT, kind="ExternalInput")
    bq_c = nc.dram_tensor("bq_c", [P, NH], F32, kind="ExternalInput")
    bk_c = nc.dram_tensor("bk_c", [P, NH], F32, kind="ExternalInput")
    bvb = nc.dram_tensor("bvb", [P, GD], F32, kind="ExternalInput")

    kp_o = nc.dram_tensor("kp", [GD, S], CDT, kind="ExternalOutput")
    vp_o = nc.dram_tensor("vp", [S, GD], CDT, kind="ExternalOutput")
    pout_o = nc.dram_tensor("pout", [S, D], CDT, kind="ExternalOutput")

    xq_r = xq_c.ap().rearrange("(c p) m -> c p m", p=P)
    xk_r = xk_c.ap().rearrange("(c p) m -> c p m", p=P)
    xv_r = xv_c.ap().rearrange("(c p) m -> c p m", p=P)
    kp_r = kp_o.ap().rearrange("(hb p) s -> hb p s", p=P)
    vp_r = vp_o.ap().rearrange("(sb p) (h d) -> sb p h d", p=P, h=NH)

    with tile.TileContext(nc) as tc, ExitStack() as top:
        const = top.enter_context(tc.tile_pool(name="const", bufs=1))
        ident = const.tile([P, P], CDT, name="ident")
        make_identity(nc, ident[:])
        bq_sb = const.tile([P, NH], F32, name="bq_sb")
        bk_sb = const.tile([P, NH], F32, name="bk_sb")
        bvb_sb = const.tile([P, NH, P], F32, name="bvb_sb")

        # long-lived activations
        persist1 = top.enter_context(tc.tile_pool(name="persist1", bufs=1))
        qpT = [persist1.tile([P, S], CDT, name=f"qpT{h}") for h in range(NH)]
        kpT = [persist1.tile([P, S], CDT, name=f"kpT{h}") for h in range(NH)]
        persist2 = top.enter_context(tc.tile_pool(name="persist2", bufs=1))
        vpB = [persist2.tile([P, NH, P + 1], CDT, name=f"vpB{sb}")
               for sb in range(NSB)]
        persist3 = top.enter_context(tc.tile_pool(name="persist3", bufs=1))
        ctxT = [persist3.tile([P, S], CDT, name=f"ctxT{h}") for h in range(NH)]
        expP = top.enter_context(tc.tile_pool(name="expP", bufs=1))
        expT = [expP.tile([P, NRING, 3 * P], CDT, name=f"expT{h}")
                for h in range(NH)]
        for sb in range(NSB):
            nc.gpsimd.memset(vpB[sb][:, :, P:P + 1], 1.0)

        # out-proj weights loaded early (DMA overlaps phase A)
        wopool = top.enter_context(tc.tile_pool(name="wopool", bufs=1))
        wo_sb = wopool.tile([P, NH, D], CDT, name="wo_sb")

        # attention working pools (SBUF)
        awk = top.enter_context(tc.tile_pool(name="awk", bufs=3))

        with ExitStack() as actx:
            wpool = actx.enter_context(tc.tile_pool(name="wpool", bufs=1))
            xpool = actx.enter_context(tc.tile_pool(name="xpool", bufs=2))
            pa = actx.enter_context(tc.tile_pool(name="pa", bufs=2,
                                                 space="PSUM"))

            # warmup: wk and the first xk chunk are SEPARATE piece-tiles
            # (dependency tracking is tile-granular) on two different DMA
            # queues (scalar=weights, sync=x) with geometric sizes, so the
            # first matmul only waits for the small piece 0 of each.
            pieces = [1, 3, 4, 8]
            poff = [0, 1, 4, 8]
            piece_of_db = [0] * 1 + [1] * 3 + [2] * 4 + [3] * 8
            wk_p = [wpool.tile([P, n, GD], CDT, name=f"wk_p{i}")
                    for i, n in enumerate(pieces)]
            xt0_p = [wpool.tile([P, n, SC], CDT, name=f"xt0_p{i}")
                     for i, n in enumerate(pieces)]
            for i, n in enumerate(pieces):
                o = poff[i]
                nc.sync.dma_start(xt0_p[i][:],
                                  xk_r[0][:, o * SC:(o + n) * SC])
                nc.scalar.dma_start(wk_p[i][:],
                                    wk_c.ap()[:, o * GD:(o + n) * GD])
            nc.scalar.dma_start(bq_sb[:], bq_c.ap())
            nc.scalar.dma_start(bk_sb[:], bk_c.ap())
            nc.scalar.dma_start(bvb_sb[:], bvb.ap())
            wv_sb = wpool.tile([P, NDB, GD], CDT, name="wv_sb")
            wq_sb = wpool.tile([P, NDB, GD], CDT, name="wq_sb")
            nc.scalar.dma_start(wv_sb[:], wv_c.ap())
            nc.scalar.dma_start(wq_sb[:], wq_c.ap())
            nc.scalar.dma_start(wo_sb[:], wo_c.ap())

            # ---- k projection (transposed): kpT[d', s] ----
            xk_t = [None] * NSC
            for sc in range(NSC):
                # prefetch next chunk BEFORE this chunk's output DMAs are
                # queued (FIFO head-of-line blocking otherwise)
                if sc == 0:
                    xk_t[1] = xpool.tile([P, NDB, SC], CDT, tag="x", name="xt")
                    nc.sync.dma_start(xk_t[1][:], xk_r[1])
                elif sc + 1 < NSC:
                    xk_t[sc + 1] = xpool.tile([P, NDB, SC], CDT, tag="x",
                                              name="xt")
                    nc.sync.dma_start(xk_t[sc + 1][:], xk_r[sc + 1])
                xt = xk_t[sc]
                ssl = slice(sc * SC, (sc + 1) * SC)
                for hb in range(NH):
                    ps = pa.tile([P, SC], F32, tag="pa", name="ps")
                    for db in range(NDB):
                        pi = piece_of_db[db]
                        dl = db - poff[pi]
                        lhsT = wk_p[pi][:, dl, hb * P:(hb + 1) * P]
                        rhs = xt0_p[pi][:, dl, :] if sc == 0 else xt[:, db, :]
                        nc.tensor.matmul(
                            ps[:],
                            lhsT=lhsT,
                            rhs=rhs,
                            start=(db == 0),
                            stop=(db == NDB - 1),
                        )
                    nc.vector.tensor_scalar_add(kpT[hb][:, ssl], ps[:],
                                                bk_sb[:, hb:hb + 1])
                    nc.scalar.dma_start(kp_r[hb][:, ssl], kpT[hb][:, ssl])

            # ---- v projection (natural layout + ones col): vpB[s, h, d'] ----
            with ExitStack() as vctx:
                pav = vctx.enter_context(tc.tile_pool(name="pav", bufs=2,
                                                      space="PSUM"))
                xv_t = [None] * (NSC + 1)
                xv_t[0] = xpool.tile([P, NDB, SC], CDT, tag="x", name="xt")
                nc.sync.dma_start(xv_t[0][:], xv_r[0])
                for sc in range(NSC):
                    if sc + 1 < NSC:
                        xv_t[sc + 1] = xpool.tile([P, NDB, SC], CDT, tag="x",
                                                  name="xt")
                        nc.sync.dma_start(xv_t[sc + 1][:], xv_r[sc + 1])
                    xt = xv_t[sc]
                    for s2 in range(SC // P):
                        sb = sc * (SC // P) + s2
                        ps2 = pav.tile([P, NH, P], F32, tag="pav", name="ps2")
                        for db in range(NDB):
                            nc.tensor.matmul(
                                ps2[:],
                                lhsT=xt[:, db, s2 * P:(s2 + 1) * P],
                                rhs=wv_sb[:, db, :],
                                start=(db == 0),
                                stop=(db == NDB - 1),
                            )
                        nc.vector.tensor_tensor(vpB[sb][:, :, 0:P], ps2[:],
                                                bvb_sb[:], AluOp.add)
                        nc.scalar.dma_start(vp_r[sb], vpB[sb][:, :, 0:P])

            # attention PSUM pools (opened after pav closes: stay <= 8 banks)
            scps = actx.enter_context(tc.tile_pool(name="scps", bufs=2,
                                                   space="PSUM"))
            pvps = actx.enter_context(tc.tile_pool(name="pvps", bufs=2,
                                                   space="PSUM"))
            trps = actx.enter_context(tc.tile_pool(name="trps", bufs=2,
                                                   space="PSUM"))

            # ---- q projection interleaved with windowed attention ----
            xq_t = [None] * NSC
            xq_t[0] = xpool.tile([P, NDB, SC], CDT, tag="x", name="xt")
            nc.sync.dma_start(xq_t[0][:], xq_r[0])
            for sc in range(NSC):
                if sc + 1 < NSC:
                    xq_t[sc + 1] = xpool.tile([P, NDB, SC], CDT, tag="x",
                                              name="xt")
                    nc.sync.dma_start(xq_t[sc + 1][:], xq_r[sc + 1])
                xt = xq_t[sc]
                ssl = slice(sc * SC, (sc + 1) * SC)
                for hb in range(NH):
                    ps = pa.tile([P, SC], F32, tag="pa", name="ps")
                    for db in range(NDB):
                        nc.tensor.matmul(
                            ps[:],
                            lhsT=wq_sb[:, db, hb * P:(hb + 1) * P],
                            rhs=xt[:, db, :],
                            start=(db == 0),
                            stop=(db == NDB - 1),
                        )
                    nc.vector.tensor_scalar_add(qpT[hb][:, ssl], ps[:],
                                                bq_sb[:, hb:hb + 1])

                # scores + exp + masks for key blocks unlocked by this chunk
                for h in range(NH):
                    for kb in _KB_BATCH[sc]:
                        nj = min(3, NSB - kb)
                        kr = kb % NRING
                        scp = scps.tile([P, 3 * P], F32, tag="sc", name="scp")
                        nc.tensor.matmul(
                            scp[:, :nj * P],
                            lhsT=kpT[h][:, kb * P:(kb + 1) * P],
                            rhs=qpT[h][:, kb * P:(kb + nj) * P],
                            start=True,
                            stop=True,
                        )
                        nc.scalar.activation(expT[h][:, kr, 0:nj * P],
                                             scp[:, :nj * P], ActFn.Exp,
                                             scale=float(SCALE))
                        # diag block: keep q >= k  (iota = qf - kp >= 0)
                        nc.gpsimd.affine_select(
                            out=expT[h][:, kr, 0:P],
                            in_=expT[h][:, kr, 0:P],
                            pattern=[[1, P]],
                            channel_multiplier=-1,
                            base=0,
                            compare_op=AluOp.is_ge,
                            fill=0.0,
                        )
                        if nj == 3:
                            # far block: keep q <= k  (iota = kp - qf >= 0)
                            nc.gpsimd.affine_select(
                                out=expT[h][:, kr, 2 * P:3 * P],
                                in_=expT[h][:, kr, 2 * P:3 * P],
                                pattern=[[-1, P]],
                                channel_multiplier=1,
                                base=0,
                                compare_op=AluOp.is_ge,
                                fill=0.0,
                            )

                # PV + normalize + transpose for query blocks now complete
                for h in range(NH):
                    for qb in _QB_BATCH[sc]:
                        kb0 = max(0, qb - 2)
                        pv = pvps.tile([P, P + 1], F32, tag="pv", name="pv")
                        for kb in range(kb0, qb + 1):
                            rel = qb - kb
                            nc.tensor.matmul(
                                pv[:],
                                lhsT=expT[h][:, kb % NRING,
                                             rel * P:(rel + 1) * P],
                                rhs=vpB[kb][:, h, :],
                                start=(kb == kb0),
                                stop=(kb == qb),
                            )
                        rinv = awk.tile([P, 1], F32, tag="rinv", name="rinv")
                        nc.vector.reciprocal(rinv[:], pv[:, P:P + 1])
                        csb = awk.tile([P, P], CDT, tag="csb", name="csb")
                        nc.vector.tensor_scalar_mul(csb[:], pv[:, 0:P],
                                                    rinv[:])
                        tp = trps.tile([P, P], CDT, tag="tp", name="tp")
                        nc.tensor.transpose(tp[:], csb[:], ident[:])
                        # spread PSUM->SBUF evictions over scalar+vector
                        if h % 2 == 0:
                            nc.scalar.copy(ctxT[h][:, qb * P:(qb + 1) * P],
                                           tp[:])
                        else:
                            nc.vector.tensor_copy(
                                ctxT[h][:, qb * P:(qb + 1) * P], tp[:])

        # ---- phase C: partial out-projection pout = ctx @ Wo_g ----
        with ExitStack() as cctx:
            cpool = cctx.enter_context(tc.tile_pool(name="cpool", bufs=2))
            psC = cctx.enter_context(tc.tile_pool(name="psC", bufs=2,
                                                  space="PSUM"))
            pout_r = pout_o.ap().rearrange("(sb p) n -> sb p n", p=P)
            for sb in range(NSB):
                po = cpool.tile([P, D], CDT, tag="po", name="po")
                for ec in range(D // 512):
                    esl = slice(ec * 512, (ec + 1) * 512)
                    psq = psC.tile([P, 512], F32, tag="psq", name="psq")
                    for h in range(NH):
                        nc.tensor.matmul(
                            psq[:],
                            lhsT=ctxT[h][:, sb * P:(sb + 1) * P],
                            rhs=wo_sb[:, h, esl],
                            start=(h == 0),
                            stop=(h == NH - 1),
                        )
                    nc.vector.tensor_copy(po[:, esl], psq[:])
                    if sb == NSB - 1:
                        nc.scalar.dma_start(pout_r[sb][:, esl], po[:, esl])
                if sb < NSB - 1:
                    eng = nc.scalar if sb % 2 else nc.sync
                    eng.dma_start(pout_r[sb], po[:])

    nc.compile()
    return nc


def kernel(q, k, v, Wq, bq, Wk, bk, Wv, bv, Wo, bo):
    global LAST_RESULTS
    import ml_dtypes

    cdt = ml_dtypes.bfloat16
    q = np.asarray(q, np.float32)
    k = np.asarray(k, np.float32)
    v = np.asarray(v, np.float32)
    Wq = np.asarray(Wq, np.float32)
    Wk = np.asarray(Wk, np.float32)
    Wv = np.asarray(Wv, np.float32)
    Wo = np.asarray(Wo, np.float32)
    bq = np.asarray(bq, np.float32)
    bk = np.asarray(bk, np.float32)
    bv = np.asarray(bv, np.float32)
    bo = np.asarray(bo, np.float32)

    if "nc" not in _CACHE:
        _CACHE["nc"] = _build_nc()
    nc = _CACHE["nc"]
    from concourse.bass_utils import run_bass_kernel_spmd

    def pack_x(x):  # [S, D] -> [(c p), (do s)] with x^T chunked along seq
        a = x.T.reshape(NDB, P, NSC, SC)
        return np.ascontiguousarray(
            a.transpose(2, 1, 0, 3).reshape(NSC * P, NDB * SC)).astype(cdt)

    def pack_w(W, gsl):  # [D, D] cols gsl -> [p, (do n)]
        a = W[:, gsl].reshape(NDB, P, GD)
        return np.ascontiguousarray(
            a.transpose(1, 0, 2).reshape(P, NDB * GD)).astype(cdt)

    def pack_wo(W, gsl):  # rows gsl -> [p, (h n)]
        a = W[gsl, :].reshape(NH, P, D)
        return np.ascontiguousarray(
            a.transpose(1, 0, 2).reshape(P, NH * D)).astype(cdt)

    xs = {}
    for b in range(B):
        xs[("q", b)] = pack_x(q[b])
        xs[("k", b)] = pack_x(k[b])
        xs[("v", b)] = pack_x(v[b])

    in_maps = []
    for core in range(8):
        b, g = divmod(core, 4)
        gsl = slice(g * GD, (g + 1) * GD)
        in_maps.append({
            "xq_c": xs[("q", b)],
            "xk_c": xs[("k", b)],
            "xv_c": xs[("v", b)],
            "wq_c": pack_w(Wq, gsl),
            "wk_c": pack_w(Wk, gsl),
            "wv_c": pack_w(Wv, gsl),
            "wo_c": pack_wo(Wo, gsl),
            "bq_c": np.ascontiguousarray(bq[gsl].reshape(NH, P).T),
            "bk_c": np.ascontiguousarray(bk[gsl].reshape(NH, P).T),
            "bvb": np.ascontiguousarray(
                np.broadcast_to(bv[gsl], (P, GD))).astype(np.float32),
        })

    trace = os.environ.get("KERNEL_TRACE", "0") == "1"
    res = run_bass_kernel_spmd(nc, in_maps, core_ids=list(range(8)),
                               trace=trace)
    LAST_RESULTS = res

    out = np.zeros((B, S, D), np.float64)
    kp = np.empty((B, S, D), np.float32)
    vp = np.empty((B, S, D), np.float32)
    for core in range(8):
        b, g = divmod(core, 4)
        gsl = slice(g * GD, (g + 1) * GD)
        r = res.results[core]
        kp[b][:, gsl] = r["kp"].astype(np.float32).T
        vp[b][:, gsl] = r["vp"].astype(np.float32)
        out[b] += r["pout"].astype(np.float64)
    out = (out + bo.astype(np.float64)).astype(np.float32)
    return out, kp, vp


# revision 23
# speedup vs baseline: 1.0215x; 1.0215x over previous
"""Trainium2 Bass kernel for a sparse (sliding-window) attention layer.

Reference computation (B=2, S=2048, D=2048, H=16 heads, window=256, fp32):
    qp = q @ Wq + bq ; kp = k @ Wk + bk ; vp = v @ Wv + bv
    per-head scores with mask (0 <= q_idx - k_idx <= 256), softmax, ctx
    out = merge_heads(ctx) @ Wo + bo
    returns (out, kp, vp)

Sharding: 8 cores = 2 (batch) x 4 (head groups of 4 heads / 512 dims).
Each core computes its batch's projections for its 512 output dims,
windowed attention for its 4 heads, and a partial out-projection (rows
of Wo owned by its heads).  Host sums the 4 bf16 partial outputs per
batch and concatenates kp/vp slices.

Attention is computed in transposed form: S^T[k, q] = kpT^T @ qpT per
key block (one N=384 matmul covering the 3 query blocks in this key
block's window), exp on the scalar engine, triangular masking with
gpsimd affine_select (zero-fill), and PV as ctx[q, dh] with lhsT =
exp^T blocks; a ones-column appended to v yields the softmax row sums
in the same matmul.  This removes all probability transposes of the
previous design; only one [128,128] ctx transpose per query block
remains (to feed the out-projection).

Matmul operands are bf16 (fp32 PSUM accumulation); outputs are written
bf16 and widened on host (tolerance 2e-2 >> bf16 rounding).
"""

import os

import numpy as np

B = 2
S = 2048
D = 2048
GD = 512          # dims per core (4 heads x 128)
NH = 4            # heads per core
P = 128
WIN = 256         # sliding window
NDB = D // P      # 16 contraction blocks
SC = 512          # seq chunk for projections
NSC = S // SC     # 4
NSB = S // P      # 16 seq blocks
NRING = 8         # exp ring slots per head
SCALE = 1.0 / np.sqrt(P)

_CACHE = {}
LAST_RESULTS = None

# attention blocks that become computable after q-projection chunk c:
# scores(kb) needs qpT blocks kb..kb+2 ; PV(qb) needs exp(kb=qb) => qb+2
_KB_BATCH = [range(0, 2), range(2, 6), range(6, 10), range(10, 16)]
_QB_BATCH = [range(0, 2), range(2, 6), range(6, 10), range(10, 16)]


def _build_nc():
    import sys
    sys.path.insert(0, "/opt/trn_rl_repo")
    import concourse.bass as bass  # noqa: F401
    import concourse.tile as tile
    from concourse import mybir, bacc
    from concourse.masks import make_identity
    from contextlib import ExitStack

    F32 = mybir.dt.float32
    CDT = mybir.dt.bfloat16
    AluOp = mybir.AluOpType
    ActFn = mybir.ActivationFunctionType

    nc = bacc.Bacc("TRN2", target_bir_lowering=False, debug=False, num_devices=8)

    # host-packed inputs: x*_c[(c p), (do s)] = x^T[do*128+p, c*512+s]
    xq_c = nc.dram_tensor("xq_c", [NSC * P, NDB * SC], CDT, kind="ExternalInput")
    xk_c = nc.dram_tensor("xk_c", [NSC * P, NDB * SC], CDT, kind="ExternalInput")
    xv_c = nc.dram_tensor("xv_c", [NSC * P, NDB * SC], CDT, kind="ExternalInput")
    # w*_c[p, (do n)] = W[do*128+p, g*512+n]
    wq_c = nc.dram_tensor("wq_c", [P, NDB * GD], CDT, kind="ExternalInput")
    wk_c = nc.dram_tensor("wk_c", [P, NDB * GD], CDT, kind="ExternalInput")
    wv_c = nc.dram_tensor("wv_c", [P, NDB * GD], CDT, kind="ExternalInput")
    # wo_c[p, (h n)] = Wo[g*512 + h*128 + p, n]
    wo_c = nc.dram_tensor("wo_c", [P, NH * D], CDT, kind="ExternalInput")
    bq_c = nc.dram_tensor("bq_c", [P, NH], F32, kind="ExternalInput")
    bk_c = nc.dram_tensor("bk_c", [P, NH], F32, kind="ExternalInput")
    bvb = nc.dram_tensor("bvb", [P, GD], F32, kind="ExternalInput")

    kp_o = nc.dram_tensor("kp", [GD, S], CDT, kind="ExternalOutput")
    vp_o = nc.dram_tensor("vp", [S, GD], CDT, kind="ExternalOutput")
    pout_o = nc.dram_tensor("pout", [S, D], CDT, kind="ExternalOutput")

    xq_r = xq_c.ap().rearrange("(c p) m -> c p m", p=P)
    xk_r = xk_c.ap().rearrange("(c p) m -> c p m", p=P)
    xv_r = xv_c.ap().rearrange("(c p) m -> c p m", p=P)
    kp_r = kp_o.ap().rearrange("(hb p) s -> hb p s", p=P)
    vp_r = vp_o.ap().rearrange("(sb p) (h d) -> sb p h d", p=P, h=NH)

    with tile.TileContext(nc) as tc, ExitStack() as top:
        const = top.enter_context(tc.tile_pool(name="const", bufs=1))
        ident = const.tile([P, P], CDT, name="ident")
        make_identity(nc, ident[:])
        bq_sb = const.tile([P, NH], F32, name="bq_sb")
        bk_sb = const.tile([P, NH], F32, name="bk_sb")
        bvb_sb = const.tile([P, NH, P], F32, name="bvb_sb")

        # long-lived activations
        persist1 = top.enter_context(tc.tile_pool(name="persist1", bufs=1))
        qpT = [persist1.tile([P, S], CDT, name=f"qpT{h}") for h in range(NH)]
        kpT = [persist1.tile([P, S], CDT, name=f"kpT{h}") for h in range(NH)]
        persist2 = top.enter_context(tc.tile_pool(name="persist2", bufs=1))
        vpB = [persist2.tile([P, NH, P + 1], CDT, name=f"vpB{sb}")
               for sb in range(NSB)]
        persist3 = top.enter_context(tc.tile_pool(name="persist3", bufs=1))
        ctxT = [persist3.tile([P, S], CDT, name=f"ctxT{h}") for h in range(NH)]
        expP = top.enter_context(tc.tile_pool(name="expP", bufs=1))
        expT = [expP.tile([P, NRING, 3 * P], CDT, name=f"expT{h}")
                for h in range(NH)]
        for sb in range(NSB):
            nc.gpsimd.memset(vpB[sb][:, :, P:P + 1], 1.0)

        # out-proj weights loaded early (DMA overlaps phase A)
        wopool = top.enter_context(tc.tile_pool(name="wopool", bufs=1))
        wo_sb = wopool.tile([P, NH, D], CDT, name="wo_sb")

        # attention working pools (SBUF)
        awk = top.enter_context(tc.tile_pool(name="awk", bufs=3))

        with ExitStack() as actx:
            wpool = actx.enter_context(tc.tile_pool(name="wpool", bufs=1))
            xpool = actx.enter_context(tc.tile_pool(name="xpool", bufs=2))
            pa = actx.enter_context(tc.tile_pool(name="pa", bufs=2,
                                                 space="PSUM"))

            # warmup: wk and the first xk chunk are SEPARATE piece-tiles
            # (dependency tracking is tile-granular) on two different DMA
            # queues (scalar=weights, sync=x) with geometric sizes, so the
            # first matmul only waits for the small piece 0 of each.
            pieces = [1, 3, 4, 8]
            poff = [0, 1, 4, 8]
            piece_of_db = [0] * 1 + [1] * 3 + [2] * 4 + [3] * 8
            wk_p = [wpool.tile([P, n, GD], CDT, name=f"wk_p{i}")
                    for i, n in enumerate(pieces)]
            xt0_p = [wpool.tile([P, n, SC], CDT, name=f"xt0_p{i}")
                     for i, n in enumerate(pieces)]
            for i, n in enumerate(pieces):
                o = poff[i]
                nc.sync.dma_start(xt0_p[i][:],
                                  xk_r[0][:, o * SC:(o + n) * SC])
                nc.scalar.dma_start(wk_p[i][:],
                                    wk_c.ap()[:, o * GD:(o + n) * GD])
                if i == 0:
                    nc.scalar.dma_start(bk_sb[:], bk_c.ap())
                    nc.scalar.dma_start(bq_sb[:], bq_c.ap())
                    nc.scalar.dma_start(bvb_sb[:], bvb.ap())
            wv_sb = wpool.tile([P, NDB, GD], CDT, name="wv_sb")
            wq_sb = wpool.tile([P, NDB, GD], CDT, name="wq_sb")
            nc.scalar.dma_start(wv_sb[:], wv_c.ap())
            nc.scalar.dma_start(wq_sb[:], wq_c.ap())
            nc.scalar.dma_start(wo_sb[:], wo_c.ap())

            # ---- k projection (transposed): kpT[d', s] ----
            xk_t = [None] * NSC
            for sc in range(NSC):
                # prefetch next chunk BEFORE this chunk's output DMAs are
                # queued (FIFO head-of-line blocking otherwise)
                if sc == 0:
                    xk_t[1] = xpool.tile([P, NDB, SC], CDT, tag="x", name="xt")
                    nc.sync.dma_start(xk_t[1][:], xk_r[1])
                elif sc + 1 < NSC:
                    xk_t[sc + 1] = xpool.tile([P, NDB, SC], CDT, tag="x",
                                              name="xt")
                    nc.sync.dma_start(xk_t[sc + 1][:], xk_r[sc + 1])
                xt = xk_t[sc]
                ssl = slice(sc * SC, (sc + 1) * SC)
                if sc == 0:
                    # db-outer with 4 concurrent PSUM groups: each arriving
                    # warmup piece immediately feeds 4 matmuls, so compute
                    # streams behind the piece DMAs instead of stalling on
                    # the full tile.
                    with ExitStack() as wctx:
                        pa0 = wctx.enter_context(
                            tc.tile_pool(name="pa0", bufs=4, space="PSUM"))
                        ps0 = [pa0.tile([P, SC], F32, tag="pa0", name="ps")
                               for _ in range(NH)]
                        for db in range(NDB):
                            pi = piece_of_db[db]
                            dl = db - poff[pi]
                            for hb in range(NH):
                                nc.tensor.matmul(
                                    ps0[hb][:],
                                    lhsT=wk_p[pi][:, dl,
                                                  hb * P:(hb + 1) * P],
                                    rhs=xt0_p[pi][:, dl, :],
                                    start=(db == 0),
                                    stop=(db == NDB - 1),
                                )
                        for hb in range(NH):
                            nc.vector.tensor_scalar_add(
                                kpT[hb][:, ssl], ps0[hb][:],
                                bk_sb[:, hb:hb + 1])
                            nc.scalar.dma_start(kp_r[hb][:, ssl],
                                                kpT[hb][:, ssl])
                else:
                    for hb in range(NH):
                        ps = pa.tile([P, SC], F32, tag="pa", name="ps")
                        for db in range(NDB):
                            pi = piece_of_db[db]
                            dl = db - poff[pi]
                            nc.tensor.matmul(
                                ps[:],
                                lhsT=wk_p[pi][:, dl, hb * P:(hb + 1) * P],
                                rhs=xt[:, db, :],
                                start=(db == 0),
                                stop=(db == NDB - 1),
                            )
                        nc.vector.tensor_scalar_add(kpT[hb][:, ssl], ps[:],
                                                    bk_sb[:, hb:hb + 1])
                        nc.scalar.dma_start(kp_r[hb][:, ssl],
                                            kpT[hb][:, ssl])

            # ---- v projection (natural layout + ones col): vpB[s, h, d'] ----
            with ExitStack() as vctx:
                pav = vctx.enter_context(tc.tile_pool(name="pav", bufs=2,
                                                      space="PSUM"))
                xv_t = [None] * (NSC + 1)
                xv_t[0] = xpool.tile([P, NDB, SC], CDT, tag="x", name="xt")
                nc.sync.dma_start(xv_t[0][:], xv_r[0])
                for sc in range(NSC):
                    if sc + 1 < NSC:
                        xv_t[sc + 1] = xpool.tile([P, NDB, SC], CDT, tag="x",
                                                  name="xt")
                        nc.sync.dma_start(xv_t[sc + 1][:], xv_r[sc + 1])
                    xt = xv_t[sc]
                    for s2 in range(SC // P):
                        sb = sc * (SC // P) + s2
                        ps2 = pav.tile([P, NH, P], F32, tag="pav", name="ps2")
                        for db in range(NDB):
                            nc.tensor.matmul(
                                ps2[:],
                                lhsT=xt[:, db, s2 * P:(s2 + 1) * P],
                                rhs=wv_sb[:, db, :],
                                start=(db == 0),
                                stop=(db == NDB - 1),
                            )
                        nc.vector.tensor_tensor(vpB[sb][:, :, 0:P], ps2[:],
                                                bvb_sb[:], AluOp.add)
                        nc.scalar.dma_start(vp_r[sb], vpB[sb][:, :, 0:P])

            # attention PSUM pools (opened after pav closes: stay <= 8 banks)
            scps = actx.enter_context(tc.tile_pool(name="scps", bufs=2,
                                                   space="PSUM"))
            pvps = actx.enter_context(tc.tile_pool(name="pvps", bufs=2,
                                                   space="PSUM"))
            trps = actx.enter_context(tc.tile_pool(name="trps", bufs=2,
                                                   space="PSUM"))

            # ---- q projection interleaved with windowed attention ----
            xq_t = [None] * NSC
            xq_t[0] = xpool.tile([P, NDB, SC], CDT, tag="x", name="xt")
            nc.sync.dma_start(xq_t[0][:], xq_r[0])
            for sc in range(NSC):
                if sc + 1 < NSC:
                    xq_t[sc + 1] = xpool.tile([P, NDB, SC], CDT, tag="x",
                                              name="xt")
                    nc.sync.dma_start(xq_t[sc + 1][:], xq_r[sc + 1])
                xt = xq_t[sc]
                ssl = slice(sc * SC, (sc + 1) * SC)
                for hb in range(NH):
                    ps = pa.tile([P, SC], F32, tag="pa", name="ps")
                    for db in range(NDB):
                        nc.tensor.matmul(
                            ps[:],
                            lhsT=wq_sb[:, db, hb * P:(hb + 1) * P],
                            rhs=xt[:, db, :],
                            start=(db == 0),
                            stop=(db == NDB - 1),
                        )
                    nc.vector.tensor_scalar_add(qpT[hb][:, ssl], ps[:],
                                                bq_sb[:, hb:hb + 1])

                # scores + exp + masks for key blocks unlocked by this chunk
                for h in range(NH):
                    for kb in _KB_BATCH[sc]:
                        nj = min(3, NSB - kb)
                        kr = kb % NRING
                        scp = scps.tile([P, 3 * P], F32, tag="sc", name="scp")
                        nc.tensor.matmul(
                            scp[:, :nj * P],
                            lhsT=kpT[h][:, kb * P:(kb + 1) * P],
                            rhs=qpT[h][:, kb * P:(kb + nj) * P],
                            start=True,
                            stop=True,
                        )
                        nc.scalar.activation(expT[h][:, kr, 0:nj * P],
                                             scp[:, :nj * P], ActFn.Exp,
                                             scale=float(SCALE))
                        # diag block: keep q >= k  (iota = qf - kp >= 0)
                        nc.gpsimd.affine_select(
                            out=expT[h][:, kr, 0:P],
                            in_=expT[h][:, kr, 0:P],
                            pattern=[[1, P]],
                            channel_multiplier=-1,
                            base=0,
                            compare_op=AluOp.is_ge,
                            fill=0.0,
                        )
                        if nj == 3:
                            # far block: keep q <= k  (iota = kp - qf >= 0)
                            nc.gpsimd.affine_select(
                                out=expT[h][:, kr, 2 * P:3 * P],
                                in_=expT[h][:, kr, 2 * P:3 * P],
                                pattern=[[-1, P]],
                                channel_multiplier=1,
                                base=0,
                                compare_op=AluOp.is_ge,
                                fill=0.0,
                            )

                # PV + normalize + transpose for query blocks now complete
                for h in range(NH):
                    for qb in _QB_BATCH[sc]:
                        kb0 = max(0, qb - 2)
                        pv = pvps.tile([P, P + 1], F32, tag="pv", name="pv")
                        for kb in range(kb0, qb + 1):
                            rel = qb - kb
                            nc.tensor.matmul(
                                pv[:],
                                lhsT=expT[h][:, kb % NRING,
                                             rel * P:(rel + 1) * P],
                                rhs=vpB[kb][:, h, :],
                                start=(kb == kb0),
                                stop=(kb == qb),
                            )
                        rinv = awk.tile([P, 1], F32, tag="rinv", name="rinv")
                        nc.vector.reciprocal(rinv[:], pv[:, P:P + 1])
                        csb = awk.tile([P, P], CDT, tag="csb", name="csb")
                        nc.vector.tensor_scalar_mul(csb[:], pv[:, 0:P],
                                                    rinv[:])
                        tp = trps.tile([P, P], CDT, tag="tp", name="tp")
                        nc.tensor.transpose(tp[:], csb[:], ident[:])
                        # spread PSUM->SBUF evictions over scalar+vector
                        if h % 2 == 0:
                            nc.scalar.copy(ctxT[h][:, qb * P:(qb + 1) * P],
                                           tp[:])
                        else:
                            nc.vector.tensor_copy(
                                ctxT[h][:, qb * P:(qb + 1) * P], tp[:])

        # ---- phase C: partial out-projection pout = ctx @ Wo_g ----
        with ExitStack() as cctx:
            cpool = cctx.enter_context(tc.tile_pool(name="cpool", bufs=2))
            psC = cctx.enter_context(tc.tile_pool(name="psC", bufs=2,
                                                  space="PSUM"))
            pout_r = pout_o.ap().rearrange("(sb p) n -> sb p n", p=P)
            for sb in range(NSB):
                po = cpool.tile([P, D], CDT, tag="po", name="po")
                for ec in range(D // 512):
                    esl = slice(ec * 512, (ec + 1) * 512)
                    psq = psC.tile([P, 512], F32, tag="psq", name="psq")
                    for h in range(NH):
                        nc.tensor.matmul(
                            psq[:],
                            lhsT=ctxT[h][:, sb * P:(sb + 1) * P],
                            rhs=wo_sb[:, h, esl],
                            start=(h == 0),
                            stop=(h == NH - 1),
                        )
                    nc.vector.tensor_copy(po[:, esl], psq[:])
                    if sb == NSB - 1:
                        nc.scalar.dma_start(pout_r[sb][:, esl], po[:, esl])
                if sb < NSB - 1:
                    eng = nc.scalar if sb % 2 else nc.sync
                    eng.dma_start(pout_r[sb], po[:])

    nc.compile()
    return nc


def kernel(q, k, v, Wq, bq, Wk, bk, Wv, bv, Wo, bo):
    global LAST_RESULTS
    import ml_dtypes

    cdt = ml_dtypes.bfloat16
    q = np.asarray(q, np.float32)
    k = np.asarray(k, np.float32)
    v = np.asarray(v, np.float32)
    Wq = np.asarray(Wq, np.float32)
    Wk = np.asarray(Wk, np.float32)
    Wv = np.asarray(Wv, np.float32)
    Wo = np.asarray(Wo, np.float32)
    bq = np.asarray(bq, np.float32)
    bk = np.asarray(bk, np.float32)
    bv = np.asarray(bv, np.float32)
    bo = np.asarray(bo, np.float32)

    if "nc" not in _CACHE:
        _CACHE["nc"] = _build_nc()
    nc = _CACHE["nc"]
    from concourse.bass_utils import run_bass_kernel_spmd

    def pack_x(x):  # [S, D] -> [(c p), (do s)] with x^T chunked along seq
        a = x.T.reshape(NDB, P, NSC, SC)
        return np.ascontiguousarray(
            a.transpose(2, 1, 0, 3).reshape(NSC * P, NDB * SC)).astype(cdt)

    def pack_w(W, gsl):  # [D, D] cols gsl -> [p, (do n)]
        a = W[:, gsl].reshape(NDB, P, GD)
        return np.ascontiguousarray(
            a.transpose(1, 0, 2).reshape(P, NDB * GD)).astype(cdt)

    def pack_wo(W, gsl):  # rows gsl -> [p, (h n)]
        a = W[gsl, :].reshape(NH, P, D)
        return np.ascontiguousarray(
            a.transpose(1, 0, 2).reshape(P, NH * D)).astype(cdt)

    xs = {}
    for b in range(B):
        xs[("q", b)] = pack_x(q[b])
        xs[("k", b)] = pack_x(k[b])
        xs[("v", b)] = pack_x(v[b])

    in_maps = []
    for core in range(8):
        b, g = divmod(core, 4)
        gsl = slice(g * GD, (g + 1) * GD)
        in_maps.append({
            "xq_c": xs[("q", b)],
            "xk_c": xs[("k", b)],
            "xv_c": xs[("v", b)],
            "wq_c": pack_w(Wq, gsl),
            "wk_c": pack_w(Wk, gsl),
            "wv_c": pack_w(Wv, gsl),
            "wo_c": pack_wo(Wo, gsl),
            "bq_c": np.ascontiguousarray(bq[gsl].reshape(NH, P).T),
            "bk_c": np.ascontiguousarray(bk[gsl].reshape(NH, P).T),
            "bvb": np.ascontiguousarray(
                np.broadcast_to(bv[gsl], (P, GD))).astype(np.float32),
        })

    trace = os.environ.get("KERNEL_TRACE", "0") == "1"
    res = run_bass_kernel_spmd(nc, in_maps, core_ids=list(range(8)),
                               trace=trace)
    LAST_RESULTS = res

    out = np.zeros((B, S, D), np.float64)
    kp = np.empty((B, S, D), np.float32)
    vp = np.empty((B, S, D), np.float32)
    for core in range(8):
        b, g = divmod(core, 4)
        gsl = slice(g * GD, (g + 1) * GD)
        r = res.results[core]
        kp[b][:, gsl] = r["kp"].astype(np.float32).T
        vp[b][:, gsl] = r["vp"].astype(np.float32)
        out[b] += r["pout"].astype(np.float64)
    out = (out + bo.astype(np.float64)).astype(np.float32)
    return out, kp, vp


# revision 24
# speedup vs baseline: 1.0410x; 1.0191x over previous
"""Trainium2 Bass kernel for a sparse (sliding-window) attention layer.

Reference computation (B=2, S=2048, D=2048, H=16 heads, window=256, fp32):
    qp = q @ Wq + bq ; kp = k @ Wk + bk ; vp = v @ Wv + bv
    per-head scores with mask (0 <= q_idx - k_idx <= 256), softmax, ctx
    out = merge_heads(ctx) @ Wo + bo
    returns (out, kp, vp)

Sharding: 8 cores = 2 (batch) x 4 (head groups of 4 heads / 512 dims).
Each core computes its batch's projections for its 512 output dims,
windowed attention for its 4 heads, and a partial out-projection (rows
of Wo owned by its heads).  Host sums the 4 bf16 partial outputs per
batch and concatenates kp/vp slices.

Attention is computed in transposed form: S^T[k, q] = kpT^T @ qpT per
key block (one N=384 matmul covering the 3 query blocks in this key
block's window), exp on the scalar engine, triangular masking with
gpsimd affine_select (zero-fill), and PV as ctx[q, dh] with lhsT =
exp^T blocks; a ones-column appended to v yields the softmax row sums
in the same matmul.  This removes all probability transposes of the
previous design; only one [128,128] ctx transpose per query block
remains (to feed the out-projection).

Matmul operands are bf16 (fp32 PSUM accumulation); outputs are written
bf16 and widened on host (tolerance 2e-2 >> bf16 rounding).
"""

import os

import numpy as np

B = 2
S = 2048
D = 2048
GD = 512          # dims per core (4 heads x 128)
NH = 4            # heads per core
P = 128
WIN = 256         # sliding window
NDB = D // P      # 16 contraction blocks
SC = 512          # seq chunk for projections
NSC = S // SC     # 4
NSB = S // P      # 16 seq blocks
NRING = 8         # exp ring slots per head
SCALE = 1.0 / np.sqrt(P)

_CACHE = {}
LAST_RESULTS = None

# attention blocks that become computable after q-projection chunk c:
# scores(kb) needs qpT blocks kb..kb+2 ; PV(qb) needs exp(kb=qb) => qb+2
_KB_BATCH = [range(0, 2), range(2, 6), range(6, 10), range(10, 16)]
_QB_BATCH = [range(0, 2), range(2, 6), range(6, 10), range(10, 16)]


def _build_nc():
    import sys
    sys.path.insert(0, "/opt/trn_rl_repo")
    import concourse.bass as bass  # noqa: F401
    import concourse.tile as tile
    from concourse import mybir, bacc
    from concourse.masks import make_identity
    from contextlib import ExitStack

    F32 = mybir.dt.float32
    CDT = mybir.dt.bfloat16
    AluOp = mybir.AluOpType
    ActFn = mybir.ActivationFunctionType

    nc = bacc.Bacc("TRN2", target_bir_lowering=False, debug=False, num_devices=8)

    # host-packed inputs: x*_c[(c p), (do s)] = x^T[do*128+p, c*512+s]
    xq_c = nc.dram_tensor("xq_c", [NSC * P, NDB * SC], CDT, kind="ExternalInput")
    xk_c = nc.dram_tensor("xk_c", [NSC * P, NDB * SC], CDT, kind="ExternalInput")
    xv_c = nc.dram_tensor("xv_c", [NSC * P, NDB * SC], CDT, kind="ExternalInput")
    # w*_c[p, (do n)] = W[do*128+p, g*512+n]
    wq_c = nc.dram_tensor("wq_c", [P, NDB * GD], CDT, kind="ExternalInput")
    wk_c = nc.dram_tensor("wk_c", [P, NDB * GD], CDT, kind="ExternalInput")
    wv_c = nc.dram_tensor("wv_c", [P, NDB * GD], CDT, kind="ExternalInput")
    # wo_c[p, (h n)] = Wo[g*512 + h*128 + p, n]
    wo_c = nc.dram_tensor("wo_c", [P, NH * D], CDT, kind="ExternalInput")
    bq_c = nc.dram_tensor("bq_c", [P, NH], F32, kind="ExternalInput")
    bk_c = nc.dram_tensor("bk_c", [P, NH], F32, kind="ExternalInput")
    bvb = nc.dram_tensor("bvb", [P, GD], F32, kind="ExternalInput")

    kp_o = nc.dram_tensor("kp", [GD, S], CDT, kind="ExternalOutput")
    vp_o = nc.dram_tensor("vp", [S, GD], CDT, kind="ExternalOutput")
    pout_o = nc.dram_tensor("pout", [S, D], CDT, kind="ExternalOutput")

    xq_r = xq_c.ap().rearrange("(c p) m -> c p m", p=P)
    xk_r = xk_c.ap().rearrange("(c p) m -> c p m", p=P)
    xv_r = xv_c.ap().rearrange("(c p) m -> c p m", p=P)
    kp_r = kp_o.ap().rearrange("(hb p) s -> hb p s", p=P)
    vp_r = vp_o.ap().rearrange("(sb p) (h d) -> sb p h d", p=P, h=NH)

    with tile.TileContext(nc) as tc, ExitStack() as top:
        const = top.enter_context(tc.tile_pool(name="const", bufs=1))
        ident = const.tile([P, P], CDT, name="ident")
        make_identity(nc, ident[:])
        bq_sb = const.tile([P, NH], F32, name="bq_sb")
        bk_sb = const.tile([P, NH], F32, name="bk_sb")
        bvb_sb = const.tile([P, NH, P], F32, name="bvb_sb")

        # long-lived activations
        persist1 = top.enter_context(tc.tile_pool(name="persist1", bufs=1))
        qpT = [persist1.tile([P, S], CDT, name=f"qpT{h}") for h in range(NH)]
        kpT = [persist1.tile([P, S], CDT, name=f"kpT{h}") for h in range(NH)]
        persist2 = top.enter_context(tc.tile_pool(name="persist2", bufs=1))
        vpB = [persist2.tile([P, NH, P + 1], CDT, name=f"vpB{sb}")
               for sb in range(NSB)]
        persist3 = top.enter_context(tc.tile_pool(name="persist3", bufs=1))
        ctxT = [persist3.tile([P, S], CDT, name=f"ctxT{h}") for h in range(NH)]
        expP = top.enter_context(tc.tile_pool(name="expP", bufs=1))
        expT = [expP.tile([P, NRING, 3 * P], CDT, name=f"expT{h}")
                for h in range(NH)]
        for sb in range(NSB):
            nc.gpsimd.memset(vpB[sb][:, :, P:P + 1], 1.0)

        # out-proj weights loaded early (DMA overlaps phase A)
        wopool = top.enter_context(tc.tile_pool(name="wopool", bufs=1))
        wo_sb = wopool.tile([P, NH, D], CDT, name="wo_sb")

        # attention working pools (SBUF)
        awk = top.enter_context(tc.tile_pool(name="awk", bufs=3))

        with ExitStack() as actx:
            wpool = actx.enter_context(tc.tile_pool(name="wpool", bufs=1))
            xpool = actx.enter_context(tc.tile_pool(name="xpool", bufs=2))
            pa = actx.enter_context(tc.tile_pool(name="pa", bufs=2,
                                                 space="PSUM"))

            # warmup: wk and the first xk chunk are SEPARATE piece-tiles
            # (dependency tracking is tile-granular) on two different DMA
            # queues (scalar=weights, sync=x) with geometric sizes, so the
            # first matmul only waits for the small piece 0 of each.
            pieces = [1, 1, 2, 4, 8]
            poff = [0, 1, 2, 4, 8]
            piece_of_db = [0, 1, 2, 2, 3, 3, 3, 3] + [4] * 8
            wk_p = [wpool.tile([P, n, GD], CDT, name=f"wk_p{i}")
                    for i, n in enumerate(pieces)]
            xt0_p = [wpool.tile([P, n, SC], CDT, name=f"xt0_p{i}")
                     for i, n in enumerate(pieces)]
            for i, n in enumerate(pieces):
                o = poff[i]
                nc.sync.dma_start(xt0_p[i][:],
                                  xk_r[0][:, o * SC:(o + n) * SC])
                nc.scalar.dma_start(wk_p[i][:],
                                    wk_c.ap()[:, o * GD:(o + n) * GD])

            nc.scalar.dma_start(bk_sb[:], bk_c.ap())
            nc.scalar.dma_start(bq_sb[:], bq_c.ap())
            nc.scalar.dma_start(bvb_sb[:], bvb.ap())
            wv_sb = wpool.tile([P, NDB, GD], CDT, name="wv_sb")
            wq_sb = wpool.tile([P, NDB, GD], CDT, name="wq_sb")
            nc.scalar.dma_start(wv_sb[:], wv_c.ap())
            nc.scalar.dma_start(wq_sb[:], wq_c.ap())
            nc.scalar.dma_start(wo_sb[:], wo_c.ap())

            # ---- k projection (transposed): kpT[d', s] ----
            xk_t = [None] * NSC
            for sc in range(NSC):
                # prefetch next chunk BEFORE this chunk's output DMAs are
                # queued (FIFO head-of-line blocking otherwise)
                if sc == 0:
                    xk_t[1] = xpool.tile([P, NDB, SC], CDT, tag="x", name="xt")
                    nc.sync.dma_start(xk_t[1][:], xk_r[1])
                elif sc + 1 < NSC:
                    xk_t[sc + 1] = xpool.tile([P, NDB, SC], CDT, tag="x",
                                              name="xt")
                    nc.sync.dma_start(xk_t[sc + 1][:], xk_r[sc + 1])
                xt = xk_t[sc]
                ssl = slice(sc * SC, (sc + 1) * SC)
                if sc == 0:
                    # db-outer with 4 concurrent PSUM groups: each arriving
                    # warmup piece immediately feeds 4 matmuls, so compute
                    # streams behind the piece DMAs instead of stalling on
                    # the full tile.
                    with ExitStack() as wctx:
                        pa0 = wctx.enter_context(
                            tc.tile_pool(name="pa0", bufs=4, space="PSUM"))
                        ps0 = [pa0.tile([P, SC], F32, tag="pa0", name="ps")
                               for _ in range(NH)]
                        for db in range(NDB):
                            pi = piece_of_db[db]
                            dl = db - poff[pi]
                            for hb in range(NH):
                                nc.tensor.matmul(
                                    ps0[hb][:],
                                    lhsT=wk_p[pi][:, dl,
                                                  hb * P:(hb + 1) * P],
                                    rhs=xt0_p[pi][:, dl, :],
                                    start=(db == 0),
                                    stop=(db == NDB - 1),
                                )
                        for hb in range(NH):
                            nc.vector.tensor_scalar_add(
                                kpT[hb][:, ssl], ps0[hb][:],
                                bk_sb[:, hb:hb + 1])
                            nc.scalar.dma_start(kp_r[hb][:, ssl],
                                                kpT[hb][:, ssl])
                else:
                    for hb in range(NH):
                        ps = pa.tile([P, SC], F32, tag="pa", name="ps")
                        for db in range(NDB):
                            pi = piece_of_db[db]
                            dl = db - poff[pi]
                            nc.tensor.matmul(
                                ps[:],
                                lhsT=wk_p[pi][:, dl, hb * P:(hb + 1) * P],
                                rhs=xt[:, db, :],
                                start=(db == 0),
                                stop=(db == NDB - 1),
                            )
                        nc.vector.tensor_scalar_add(kpT[hb][:, ssl], ps[:],
                                                    bk_sb[:, hb:hb + 1])
                        nc.scalar.dma_start(kp_r[hb][:, ssl],
                                            kpT[hb][:, ssl])

            # ---- v projection (natural layout + ones col): vpB[s, h, d'] ----
            with ExitStack() as vctx:
                pav = vctx.enter_context(tc.tile_pool(name="pav", bufs=2,
                                                      space="PSUM"))
                xv_t = [None] * (NSC + 1)
                xv_t[0] = xpool.tile([P, NDB, SC], CDT, tag="x", name="xt")
                nc.sync.dma_start(xv_t[0][:], xv_r[0])
                for sc in range(NSC):
                    if sc + 1 < NSC:
                        xv_t[sc + 1] = xpool.tile([P, NDB, SC], CDT, tag="x",
                                                  name="xt")
                        nc.sync.dma_start(xv_t[sc + 1][:], xv_r[sc + 1])
                    xt = xv_t[sc]
                    for s2 in range(SC // P):
                        sb = sc * (SC // P) + s2
                        ps2 = pav.tile([P, NH, P], F32, tag="pav", name="ps2")
                        for db in range(NDB):
                            nc.tensor.matmul(
                                ps2[:],
                                lhsT=xt[:, db, s2 * P:(s2 + 1) * P],
                                rhs=wv_sb[:, db, :],
                                start=(db == 0),
                                stop=(db == NDB - 1),
                            )
                        nc.vector.tensor_tensor(vpB[sb][:, :, 0:P], ps2[:],
                                                bvb_sb[:], AluOp.add)
                        nc.scalar.dma_start(vp_r[sb], vpB[sb][:, :, 0:P])

            # attention PSUM pools (opened after pav closes: stay <= 8 banks)
            scps = actx.enter_context(tc.tile_pool(name="scps", bufs=2,
                                                   space="PSUM"))
            pvps = actx.enter_context(tc.tile_pool(name="pvps", bufs=2,
                                                   space="PSUM"))
            trps = actx.enter_context(tc.tile_pool(name="trps", bufs=2,
                                                   space="PSUM"))

            # ---- q projection interleaved with windowed attention ----
            xq_t = [None] * NSC
            xq_t[0] = xpool.tile([P, NDB, SC], CDT, tag="x", name="xt")
            nc.sync.dma_start(xq_t[0][:], xq_r[0])
            for sc in range(NSC):
                if sc + 1 < NSC:
                    xq_t[sc + 1] = xpool.tile([P, NDB, SC], CDT, tag="x",
                                              name="xt")
                    nc.sync.dma_start(xq_t[sc + 1][:], xq_r[sc + 1])
                xt = xq_t[sc]
                ssl = slice(sc * SC, (sc + 1) * SC)
                for hb in range(NH):
                    ps = pa.tile([P, SC], F32, tag="pa", name="ps")
                    for db in range(NDB):
                        nc.tensor.matmul(
                            ps[:],
                            lhsT=wq_sb[:, db, hb * P:(hb + 1) * P],
                            rhs=xt[:, db, :],
                            start=(db == 0),
                            stop=(db == NDB - 1),
                        )
                    nc.vector.tensor_scalar_add(qpT[hb][:, ssl], ps[:],
                                                bq_sb[:, hb:hb + 1])

                # scores + exp + masks for key blocks unlocked by this chunk
                for h in range(NH):
                    for kb in _KB_BATCH[sc]:
                        nj = min(3, NSB - kb)
                        kr = kb % NRING
                        scp = scps.tile([P, 3 * P], F32, tag="sc", name="scp")
                        nc.tensor.matmul(
                            scp[:, :nj * P],
                            lhsT=kpT[h][:, kb * P:(kb + 1) * P],
                            rhs=qpT[h][:, kb * P:(kb + nj) * P],
                            start=True,
                            stop=True,
                        )
                        nc.scalar.activation(expT[h][:, kr, 0:nj * P],
                                             scp[:, :nj * P], ActFn.Exp,
                                             scale=float(SCALE))
                        # diag block: keep q >= k  (iota = qf - kp >= 0)
                        nc.gpsimd.affine_select(
                            out=expT[h][:, kr, 0:P],
                            in_=expT[h][:, kr, 0:P],
                            pattern=[[1, P]],
                            channel_multiplier=-1,
                            base=0,
                            compare_op=AluOp.is_ge,
                            fill=0.0,
                        )
                        if nj == 3:
                            # far block: keep q <= k  (iota = kp - qf >= 0)
                            nc.gpsimd.affine_select(
                                out=expT[h][:, kr, 2 * P:3 * P],
                                in_=expT[h][:, kr, 2 * P:3 * P],
                                pattern=[[-1, P]],
                                channel_multiplier=1,
                                base=0,
                                compare_op=AluOp.is_ge,
                                fill=0.0,
                            )

                # PV + normalize + transpose for query blocks now complete
                for h in range(NH):
                    for qb in _QB_BATCH[sc]:
                        kb0 = max(0, qb - 2)
                        pv = pvps.tile([P, P + 1], F32, tag="pv", name="pv")
                        for kb in range(kb0, qb + 1):
                            rel = qb - kb
                            nc.tensor.matmul(
                                pv[:],
                                lhsT=expT[h][:, kb % NRING,
                                             rel * P:(rel + 1) * P],
                                rhs=vpB[kb][:, h, :],
                                start=(kb == kb0),
                                stop=(kb == qb),
                            )
                        rinv = awk.tile([P, 1], F32, tag="rinv", name="rinv")
                        nc.vector.reciprocal(rinv[:], pv[:, P:P + 1])
                        csb = awk.tile([P, P], CDT, tag="csb", name="csb")
                        nc.vector.tensor_scalar_mul(csb[:], pv[:, 0:P],
                                                    rinv[:])
                        tp = trps.tile([P, P], CDT, tag="tp", name="tp")
                        nc.tensor.transpose(tp[:], csb[:], ident[:])
                        # spread PSUM->SBUF evictions over scalar+vector
                        if h % 2 == 0:
                            nc.scalar.copy(ctxT[h][:, qb * P:(qb + 1) * P],
                                           tp[:])
                        else:
                            nc.vector.tensor_copy(
                                ctxT[h][:, qb * P:(qb + 1) * P], tp[:])

        # ---- phase C: partial out-projection pout = ctx @ Wo_g ----
        with ExitStack() as cctx:
            cpool = cctx.enter_context(tc.tile_pool(name="cpool", bufs=2))
            psC = cctx.enter_context(tc.tile_pool(name="psC", bufs=2,
                                                  space="PSUM"))
            pout_r = pout_o.ap().rearrange("(sb p) n -> sb p n", p=P)
            for sb in range(NSB):
                po = cpool.tile([P, D], CDT, tag="po", name="po")
                for ec in range(D // 512):
                    esl = slice(ec * 512, (ec + 1) * 512)
                    psq = psC.tile([P, 512], F32, tag="psq", name="psq")
                    for h in range(NH):
                        nc.tensor.matmul(
                            psq[:],
                            lhsT=ctxT[h][:, sb * P:(sb + 1) * P],
                            rhs=wo_sb[:, h, esl],
                            start=(h == 0),
                            stop=(h == NH - 1),
                        )
                    nc.vector.tensor_copy(po[:, esl], psq[:])
                    if sb >= NSB - 2:
                        eng = nc.scalar if ec % 2 else nc.sync
                        eng.dma_start(pout_r[sb][:, esl], po[:, esl])
                if sb < NSB - 2:
                    eng = nc.scalar if sb % 2 else nc.sync
                    eng.dma_start(pout_r[sb], po[:])

    nc.compile()
    return nc


def kernel(q, k, v, Wq, bq, Wk, bk, Wv, bv, Wo, bo):
    global LAST_RESULTS
    import ml_dtypes

    cdt = ml_dtypes.bfloat16
    q = np.asarray(q, np.float32)
    k = np.asarray(k, np.float32)
    v = np.asarray(v, np.float32)
    Wq = np.asarray(Wq, np.float32)
    Wk = np.asarray(Wk, np.float32)
    Wv = np.asarray(Wv, np.float32)
    Wo = np.asarray(Wo, np.float32)
    bq = np.asarray(bq, np.float32)
    bk = np.asarray(bk, np.float32)
    bv = np.asarray(bv, np.float32)
    bo = np.asarray(bo, np.float32)

    if "nc" not in _CACHE:
        _CACHE["nc"] = _build_nc()
    nc = _CACHE["nc"]
    from concourse.bass_utils import run_bass_kernel_spmd

    def pack_x(x):  # [S, D] -> [(c p), (do s)] with x^T chunked along seq
        a = x.T.reshape(NDB, P, NSC, SC)
        return np.ascontiguousarray(
            a.transpose(2, 1, 0, 3).reshape(NSC * P, NDB * SC)).astype(cdt)

    def pack_w(W, gsl):  # [D, D] cols gsl -> [p, (do n)]
        a = W[:, gsl].reshape(NDB, P, GD)
        return np.ascontiguousarray(
            a.transpose(1, 0, 2).reshape(P, NDB * GD)).astype(cdt)

    def pack_wo(W, gsl):  # rows gsl -> [p, (h n)]
        a = W[gsl, :].reshape(NH, P, D)
        return np.ascontiguousarray(
            a.transpose(1, 0, 2).reshape(P, NH * D)).astype(cdt)

    xs = {}
    for b in range(B):
        xs[("q", b)] = pack_x(q[b])
        xs[("k", b)] = pack_x(k[b])
        xs[("v", b)] = pack_x(v[b])

    in_maps = []
    for core in range(8):
        b, g = divmod(core, 4)
        gsl = slice(g * GD, (g + 1) * GD)
        in_maps.append({
            "xq_c": xs[("q", b)],
            "xk_c": xs[("k", b)],
            "xv_c": xs[("v", b)],
            "wq_c": pack_w(Wq, gsl),
            "wk_c": pack_w(Wk, gsl),
            "wv_c": pack_w(Wv, gsl),
            "wo_c": pack_wo(Wo, gsl),
            "bq_c": np.ascontiguousarray(bq[gsl].reshape(NH, P).T),
            "bk_c": np.ascontiguousarray(bk[gsl].reshape(NH, P).T),
            "bvb": np.ascontiguousarray(
                np.broadcast_to(bv[gsl], (P, GD))).astype(np.float32),
        })

    trace = os.environ.get("KERNEL_TRACE", "0") == "1"
    res = run_bass_kernel_spmd(nc, in_maps, core_ids=list(range(8)),
                               trace=trace)
    LAST_RESULTS = res

    out = np.zeros((B, S, D), np.float64)
    kp = np.empty((B, S, D), np.float32)
    vp = np.empty((B, S, D), np.float32)
    for core in range(8):
        b, g = divmod(core, 4)
        gsl = slice(g * GD, (g + 1) * GD)
        r = res.results[core]
        kp[b][:, gsl] = r["kp"].astype(np.float32).T
        vp[b][:, gsl] = r["vp"].astype(np.float32)
        out[b] += r["pout"].astype(np.float64)
    out = (out + bo.astype(np.float64)).astype(np.float32)
    return out, kp, vp
